# revision 1
# baseline (speedup 1.0000x reference)
"""Fused causal multi-head attention block on 8 Trainium2 NeuronCores.

Problem (GPT-2 style attention, B=2, S=2048, D=1024, H=16, hd=64):
    qkv = x @ w_attn + b_attn ; split q,k,v ; per-head causal softmax(q k^T / 8) v
    out = attn_out @ w_proj + b_proj

Sharding: data parallel on batch (2) x tensor parallel on heads (4 groups of 4
heads). Core c -> batch c//4, head group c%4. Each core computes a partial
[S, D] output (its heads' slice of w_proj rows); host sums the 4 partials per
batch and adds b_proj.

Per-core kernel layout tricks:
- scores are computed TRANSPOSED (scoresT[key, query]) so the softmax
  denominator falls out of the attn@v matmul by appending a ones-column to v:
  [v | 1]^T @ exp(scoresT) yields the unnormalized output and the per-query
  denominator in one PSUM accumulation.
- matmul inputs are fp16 (full PE rate + fast weight loads); all accumulation
  is fp32 in PSUM. exp(s/8) is in [0, ~13], well inside fp16 range.
- causal masking: fully-masked blocks are skipped via restricted matmul
  widths; diagonal blocks get a -30000 triangle accumulated into the score
  PSUM by an identity matmul, so exp() yields exact zeros and the vector
  engine stays out of the score->attnv chain.
- emission is chunk-pipelined (QKV chunk c, attention chunk c, projection
  chunk c) so the PE always has dense matmul work while ScalarE runs exp.
"""

import sys

sys.path.insert(0, "/opt/trn_rl_repo")

import numpy as np

import concourse.bass as bass
import concourse.mybir as mybir
import concourse.tile as tile
from concourse import bacc
from concourse.bass_utils import run_bass_kernel_spmd

F32 = mybir.dt.float32
F16 = mybir.dt.float16
AFT = mybir.ActivationFunctionType

B, S, D, H, HD = 2, 2048, 1024, 16, 64
NCORES = 8
HPC = 4            # heads per core
CH = HPC * HD      # 256 channels per core
VW = HD + 1        # v width incl. ones column
P = 128
KT = D // P        # 8 contraction tiles over D
SQ = 512           # query/N chunk
NSQ = S // SQ      # 4
NST = S // P       # 16 seq tiles
SCALE = 1.0 / np.sqrt(HD)
MASKNEG = -30000.0


def emit_kernel(nc, tc, ap):
    """Emit the per-core program. `ap` is a dict of DRAM APs."""
    with (
        tc.tile_pool(name="const", bufs=1) as cp,
        tc.tile_pool(name="xw", bufs=1) as xw,
        tc.tile_pool(name="act", bufs=1) as acts,
        tc.tile_pool(name="ex", bufs=16) as exp_pool,
        tc.tile_pool(name="dh", bufs=4) as dh_pool,
        tc.tile_pool(name="rc", bufs=2) as rc_pool,
        tc.tile_pool(name="osb", bufs=3) as osb,
        tc.tile_pool(name="psA", bufs=2, space="PSUM") as psA,
        tc.tile_pool(name="psB", bufs=2, space="PSUM") as psB,
        tc.tile_pool(name="psC", bufs=2, space="PSUM") as psC,
    ):
        # ---- PE warmup: dense dummy matmuls while input DMAs stream in.
        # The PE clock-gate (HAM) unthrottles 1.2->2.4 GHz only after ~3.4us
        # of sustained matmul activity; burn that in on scratch data.
        wsrc = cp.tile([P, SQ], F16, name="wsrc", tag="wsrc")
        nc.gpsimd.memset(wsrc, 0.0)
        wps = psB.tile([P, SQ], F32, name="wps", tag="acc")
        for i in range(16):
            nc.tensor.matmul(
                wps, wsrc[:, 0:P], wsrc, start=(i == 0), stop=(i == 15),
            )

        # ---- constants ----
        tri = cp.tile([P, P], F16, name="tri", tag="tri")
        nc.sync.dma_start(tri, ap["tri"])
        bq = cp.tile([P, 2], F32, name="bq", tag="bq")
        nc.sync.dma_start(bq, ap["bq"])
        bk = cp.tile([P, 2], F32, name="bk", tag="bk")
        nc.sync.dma_start(bk, ap["bk"])
        bv = cp.tile([1, HPC * VW], F16, name="bv", tag="bv")
        nc.sync.dma_start(bv, ap["bv"])
        ones1 = cp.tile([1, P], F16, name="ones1", tag="ones1")
        nc.sync.dma_start(ones1, ap["ones1"])

        # ---- weight/x loads. Few big DMAs: each dma_start costs ~600ns of
        # serialized issue on the Sync queue, so 34 small loads would stagger
        # the late k-tiles by ~20us. Two halves for x (so the QKV k-loop can
        # start on the first half), one DMA per weight tensor.
        def kmaj(dram_ap, rows, cols):
            return dram_ap[0:rows, :].rearrange("(k p) c -> p k c", p=P)
        xts = xw.tile([P, KT, S], F16, name="xts", tag="xts")
        half = KT // 2
        nc.sync.dma_start(xts[:, 0:half, :], kmaj(ap["xT"], half * P, S))
        nc.sync.dma_start(
            xts[:, half:KT, :],
            ap["xT"][half * P:KT * P, :].rearrange("(k p) c -> p k c", p=P),
        )
        wq = xw.tile([P, KT, CH], F16, name="wq", tag="wq")
        nc.sync.dma_start(wq, kmaj(ap["wq"], KT * P, CH))
        wk = xw.tile([P, KT, CH], F16, name="wk", tag="wk")
        nc.sync.dma_start(wk, kmaj(ap["wk"], KT * P, CH))
        wv = xw.tile([P, KT, HPC * VW], F16, name="wv", tag="wv")
        nc.sync.dma_start(wv, kmaj(ap["wv"], KT * P, HPC * VW))
        wp = xw.tile([P, 2, D], F16, name="wp", tag="wp")
        nc.sync.dma_start(wp, kmaj(ap["wp"], 2 * P, D))
        xts_k = [xts[:, k, :] for k in range(KT)]
        wq_t = [wq[:, k, :] for k in range(KT)]
        wk_t = [wk[:, k, :] for k in range(KT)]
        wv_t = [wv[:, k, :] for k in range(KT)]
        wp_t = [wp[:, k, :] for k in range(2)]

        # ---- activations living across phases ----
        qT = [acts.tile([P, S], F16, name=f"qT{i}", tag=f"qT{i}") for i in range(2)]
        kTt = [acts.tile([P, S], F16, name=f"kT{i}", tag=f"kT{i}") for i in range(2)]
        vv = acts.tile([P, NST, HPC * VW], F16, name="vv", tag="vv")
        outT = [acts.tile([P, S], F16, name=f"oT{i}", tag=f"oT{i}") for i in range(2)]

        def qkv_qk_group(c, dst, wt, bias, i):
            ps = psA.tile([P, SQ], F32, name="ps", tag="ps")
            for k in range(KT):
                nc.tensor.matmul(
                    ps,
                    wt[k][:, i * P:(i + 1) * P],
                    xts_k[k][:, c * SQ:(c + 1) * SQ],
                    start=(k == 0),
                    stop=(k == KT - 1),
                )
            with nc.allow_low_precision(reason="fp16 matmul inputs"):
                nc.vector.tensor_scalar_add(
                    dst[i][:, c * SQ:(c + 1) * SQ], ps, bias[:, i:i + 1],
                )

        def qkv_v_group(st):
            # v rows (natural layout + interleaved ones cols)
            ps = psA.tile([P, SQ], F32, name="psv", tag="ps")
            psv = ps[:, 0:HPC * VW]
            for k in range(KT):
                nc.tensor.matmul(
                    psv,
                    xts_k[k][:, st * P:(st + 1) * P],
                    wv_t[k],
                    start=(k == 0),
                    stop=False,
                )
            # += ones_col(seq) x (bv | interleaved 1.0): v-bias + ones col
            nc.tensor.matmul(psv, ones1, bv, start=False, stop=True)
            with nc.allow_low_precision(reason="fp16 matmul inputs"):
                nc.vector.tensor_copy(vv[:, st, :], psv)

        def qkv_groups(c):
            for dst, wt, bias in ((qT, wq_t, bq), (kTt, wk_t, bk)):
                for i in range(2):
                    yield lambda dst=dst, wt=wt, bias=bias, i=i: \
                        qkv_qk_group(c, dst, wt, bias, i)
            for st in range(4 * c, 4 * c + 4):
                yield lambda st=st: qkv_v_group(st)

        def attention_pair(i, c, fillers=()):
            """Heads 2i (kT/qT rows 0:64) and 2i+1 (rows 64:128) together.

            Both heads' scores for a key tile land in one 2-bank PSUM tile so
            a single exp instruction covers them (halves ScalarE instruction
            count). All scores are emitted before all attnv matmuls: the PE
            stream is in-order, so this keeps the PE on scores while
            ScalarE's exps pipeline behind."""
            nkt = 4 * (c + 1)
            accs = [psB.tile([VW, SQ], F32, name="acc", tag="acc")
                    for _ in range(2)]

            exs = []
            for kt in range(nkt):
                colo = max(0, kt * P - c * SQ)
                diag = colo > 0 or kt * P == c * SQ
                sc2 = psC.tile([P, 2, SQ], F32, name="sc2", tag="sc")
                for j in range(2):
                    ro = j * 64
                    nc.tensor.matmul(
                        sc2[:, j, colo:SQ],
                        kTt[i][ro:ro + 64, kt * P:(kt + 1) * P],
                        qT[i][ro:ro + 64, c * SQ + colo:(c + 1) * SQ],
                        start=True,
                        stop=True,
                    )
                ex2 = exp_pool.tile([P, 2, SQ], F16, name="ex2", tag="ex")
                nc.scalar.activation(
                    ex2[:, :, colo:SQ], sc2[:, :, colo:SQ], AFT.Exp, scale=SCALE,
                )
                if diag:
                    # zero the masked triangle of the diagonal block; runs on
                    # VectorE well before the (much later) attnv consumers
                    nc.vector.tensor_mul(
                        ex2[:, :, colo:colo + P],
                        ex2[:, :, colo:colo + P],
                        tri[:, None, :].broadcast_to([P, 2, P]),
                    )
                exs.append((ex2, kt, colo))
            fillers = list(fillers)
            nf = len(fillers)
            for ex2, kt, colo in exs:
                for j in range(2):
                    h = 2 * i + j
                    nc.tensor.matmul(
                        accs[j][:, colo:SQ],
                        vv[:, kt, h * VW:(h + 1) * VW],
                        ex2[:, j, colo:SQ],
                        start=(kt == 0),
                        stop=(kt == nkt - 1),
                    )
                # dense PE filler between exp-paced attnv groups
                while fillers and len(fillers) > nf * (nkt - 1 - kt) // nkt:
                    fillers.pop(0)()
            dns = []
            for j in range(2):
                with nc.allow_low_precision(reason="fp16 matmul inputs"):
                    nc.vector.tensor_copy(
                        outT[i][j * 64:j * 64 + 64, c * SQ:(c + 1) * SQ],
                        accs[j][0:64, :],
                    )
                dn = dh_pool.tile([1, SQ], F16, name="dn", tag="dn")
                with nc.allow_low_precision(reason="fp16 matmul inputs"):
                    nc.vector.tensor_copy(dn, accs[j][64:65, :])
                dns.append(dn)
            return dns

        def norm_pair(c, i, dns):
            # outT *= 1/denominator: broadcast denoms via K=1 matmuls, one
            # 128-lane fast reciprocal, one fp16 multiply
            if True:
                db = psA.tile([P, SQ], F32, name="ps", tag="ps")
                nc.tensor.matmul(
                    db[0:64, :], ones1[:, 0:64], dns[0],
                    start=True, stop=True,
                )
                nc.tensor.matmul(
                    db[64:P, :], ones1[:, 0:64], dns[1],
                    start=True, stop=True,
                )
                rc32 = rc_pool.tile([P, SQ], F32, name="rc32", tag="rc32")
                nc.vector.reciprocal_approx_fast(rc32, db)
                with nc.allow_low_precision(reason="fp16 matmul inputs"):
                    nc.vector.tensor_mul(
                        outT[i][:, c * SQ:(c + 1) * SQ],
                        outT[i][:, c * SQ:(c + 1) * SQ],
                        rc32,
                    )

        def proj_mtile(m, only_kk=None, ps_list=None):
            for nch in range(2):
                if only_kk == 1:
                    ps = ps_list[nch]
                else:
                    ps = psA.tile([P, SQ], F32, name="ps", tag="ps")
                    if ps_list is not None:
                        ps_list.append(ps)
                kks = (0, 1) if only_kk is None else (only_kk,)
                for kk in kks:
                    nc.tensor.matmul(
                        ps,
                        outT[kk][:, m * P:(m + 1) * P],
                        wp_t[kk][:, nch * SQ:(nch + 1) * SQ],
                        start=(kk == 0),
                        stop=(kk == 1),
                    )
                if only_kk == 0:
                    continue
                ob = osb.tile([P, SQ], F16, name="ob", tag="ob")
                with nc.allow_low_precision(reason="partial sums; host sums fp32"):
                    nc.vector.tensor_copy(ob, ps)
                nc.sync.dma_start(
                    ap["out"][m * P:(m + 1) * P, nch * SQ:(nch + 1) * SQ], ob,
                )

        # ---- chunk-pipelined main body ----
        # chunk 0 QKV upfront, with warmup matmuls sprinkled between groups to
        # keep the PE (and its clock gate) busy while input DMAs stream in
        for gi, g in enumerate(qkv_groups(0)):
            g()
            for i in range(3):
                nc.tensor.matmul(
                    wps, wsrc[:, 0:P], wsrc,
                    start=(i == 0), stop=(i == 2),
                )
        # attention(c) runs against qkv chunks emitted one chunk ahead.
        # Dense PE filler between heads: remaining qkv chunks during c=1,2
        # and the saved-up projection tiles of chunks 0-2 during c=3 (the
        # largest, most exp-bound chunk). norm runs per head-pair so the
        # reciprocal chain starts as soon as both heads of a pair finish.
        for c in range(NSQ):
            nxt = list(qkv_groups(c + 1)) if c + 1 < NSQ else []
            for i in range(2):
                fillers = list(nxt[4 * i:4 * i + 4])
                if c == NSQ - 1:
                    fillers += [
                        (lambda m=m: proj_mtile(m))
                        for m in range(6 * i, 6 * i + 6)
                    ]
                dns = attention_pair(i, c, fillers)
                norm_pair(c, i, dns)
                if c == NSQ - 1 and i == 1:
                    # first contraction half of the last projection tiles can
                    # start as soon as outT[0] chunk 3 is normalized
                    tail_ps = {12: []}
                    proj_mtile(12, only_kk=0, ps_list=tail_ps[12])
        for m in range(4 * (NSQ - 1), 4 * NSQ):
            if m in tail_ps:
                proj_mtile(m, only_kk=1, ps_list=tail_ps[m])
            else:
                proj_mtile(m)


def build_program():
    nc = bacc.Bacc("TRN2", target_bir_lowering=False, debug=False,
                   num_devices=NCORES)
    ap = {}
    for name, shape, dt in (
        ("xT", [D, S], F16), ("wq", [D, CH], F16), ("wk", [D, CH], F16),
        ("wv", [D, HPC * VW], F16), ("bq", [P, 2], F32), ("bk", [P, 2], F32),
        ("bv", [1, HPC * VW], F16), ("wp", [CH, D], F16),
        ("tri", [P, P], F16), ("ones1", [1, P], F16),
    ):
        ap[name] = nc.dram_tensor(name, shape, dt, kind="ExternalInput").ap()
    ap["out"] = nc.dram_tensor("out", [S, D], F16, kind="ExternalOutput").ap()

    with tile.TileContext(nc) as tc:
        emit_kernel(nc, tc, ap)
    nc.compile()
    return nc


def make_core_inputs(hidden_states, w_attn, b_attn, w_proj):
    """Host-side sharding: per-core input dicts (core = batch*4 + head_group)."""
    f16, f32 = np.float16, np.float32
    x = np.asarray(hidden_states, f32)
    w_attn = np.asarray(w_attn, f32)
    b_attn = np.asarray(b_attn, f32)
    w_proj = np.asarray(w_proj, f32)

    tri = (np.arange(P)[:, None] <= np.arange(P)[None, :]).astype(f16)
    ones_row = np.ones((1, P), f16)
    xTs = [np.ascontiguousarray(x[b].T).astype(f16) for b in range(B)]

    in_maps = []
    for core in range(NCORES):
        b, g = core // HPC, core % HPC
        wq = np.ascontiguousarray(w_attn[:, g * CH:(g + 1) * CH]).astype(f16)
        wk = np.ascontiguousarray(
            w_attn[:, D + g * CH:D + (g + 1) * CH]).astype(f16)
        wv = np.zeros((D, HPC * VW), f16)
        bv = np.zeros((1, HPC * VW), f16)
        for h in range(HPC):
            src = 2 * D + (g * HPC + h) * HD
            wv[:, h * VW:h * VW + HD] = w_attn[:, src:src + HD]
            bv[0, h * VW:h * VW + HD] = b_attn[src:src + HD]
            bv[0, h * VW + HD] = 1.0
        bq = np.ascontiguousarray(
            b_attn[g * CH:(g + 1) * CH].reshape(2, P).T)
        bk = np.ascontiguousarray(
            b_attn[D + g * CH:D + (g + 1) * CH].reshape(2, P).T)
        wp = np.ascontiguousarray(w_proj[g * CH:(g + 1) * CH, :]).astype(f16)
        in_maps.append({
            "xT": xTs[b], "wq": wq, "wk": wk, "wv": wv,
            "bq": bq, "bk": bk, "bv": bv, "wp": wp,
            "tri": tri, "ones1": ones_row,
        })
    return in_maps


_PROGRAM = None


def kernel(hidden_states, w_attn, b_attn, w_proj, b_proj):
    global _PROGRAM
    if _PROGRAM is None:
        _PROGRAM = build_program()
    in_maps = make_core_inputs(hidden_states, w_attn, b_attn, w_proj)
    res = run_bass_kernel_spmd(_PROGRAM, in_maps, core_ids=list(range(NCORES)))
    out = np.zeros((B, S, D), np.float32)
    for core in range(NCORES):
        out[core // HPC] += res.results[core]["out"].astype(np.float32)
    out += np.asarray(b_proj, np.float32)
    return out



# revision 5
# speedup vs baseline: 1.1680x; 1.1680x over previous
"""Fused causal multi-head attention block on 8 Trainium2 NeuronCores.

Problem (GPT-2 style attention, B=2, S=2048, D=1024, H=16, hd=64):
    qkv = x @ w_attn + b_attn ; split q,k,v ; per-head causal softmax(q k^T / 8) v
    out = attn_out @ w_proj + b_proj

Sharding: data parallel on batch (2) x tensor parallel on heads (4 groups of 4
heads). Core c -> batch c//4, head group c%4. Each core computes a partial
[S, D] output (its heads' slice of w_proj rows); host sums the 4 partials per
batch and adds b_proj.

Per-core kernel layout tricks:
- scores are computed TRANSPOSED (scoresT[key, query]) so the softmax
  denominator falls out of the attn@v matmul by appending a ones-column to v:
  [v | 1]^T @ exp(scoresT) yields the unnormalized output and the per-query
  denominator in one PSUM accumulation.
- matmul inputs are fp16 (full PE rate + fast weight loads); all accumulation
  is fp32 in PSUM. exp(s/8) is in [0, ~13], well inside fp16 range.
- causal masking: fully-masked blocks are skipped via restricted matmul
  widths; diagonal blocks get their exp output multiplied by a 0/1 triangle
  on the (otherwise idle) GpSimd engine, keeping Vector free for psum copies.
- all DRAM tensors are host-relayouted to partition-major contiguous form so
  every DMA issue is a cheap 128-line descriptor; x is loaded per 512-query
  chunk (chunk 0 first) and weights stream in parallel on the Scalar HWDGE
  ring, so real QKV work starts ~8us in with no junk warmup needed.
- emission is chunk-pipelined (QKV chunk c, attention chunk c, projection
  chunk c) so the PE always has dense matmul work while ScalarE runs exp;
  each pair's normalization matmuls are deferred past the next pair's score
  matmuls so the in-order PE never waits on Vector/GpSimd latency.
"""

import sys

sys.path.insert(0, "/opt/trn_rl_repo")

import numpy as np

import concourse.bass as bass
import concourse.mybir as mybir
import concourse.tile as tile
from concourse import bacc
from concourse.bass_utils import run_bass_kernel_spmd

F32 = mybir.dt.float32
F16 = mybir.dt.float16
AFT = mybir.ActivationFunctionType

B, S, D, H, HD = 2, 2048, 1024, 16, 64
NCORES = 8
HPC = 4            # heads per core
CH = HPC * HD      # 256 channels per core
VW = HD + 1        # v width incl. ones column
P = 128
KT = D // P        # 8 contraction tiles over D
SQ = 512           # query/N chunk
NSQ = S // SQ      # 4
NST = S // P       # 16 seq tiles
SCALE = 1.0 / np.sqrt(HD)


def emit_kernel(nc, tc, ap):
    """Emit the per-core program. `ap` is a dict of DRAM APs."""
    with (
        tc.tile_pool(name="const", bufs=1) as cp,
        tc.tile_pool(name="xw", bufs=1) as xw,
        tc.tile_pool(name="act", bufs=1) as acts,
        tc.tile_pool(name="ex", bufs=16) as exp_pool,
        tc.tile_pool(name="dh", bufs=4) as dh_pool,
        tc.tile_pool(name="rc", bufs=2) as rc_pool,
        tc.tile_pool(name="osb", bufs=3) as osb,
        tc.tile_pool(name="psA", bufs=2, space="PSUM") as psA,
        tc.tile_pool(name="psB", bufs=2, space="PSUM") as psB,
        tc.tile_pool(name="psC", bufs=2, space="PSUM") as psC,
    ):
        # ---- input DMAs. Two parallel HWDGE rings: x chunks + small consts
        # on Sync, weights on Scalar. Chunk-0 x and wq arrive ~8us in so the
        # PE starts real QKV work immediately (no junk warmup).
        xts = xw.tile([P, NSQ, KT, SQ], F16, name="xts", tag="xts")
        nc.sync.dma_start(xts[:, 0], ap["xln"][:, 0])
        tri = cp.tile([P, P], F16, name="tri", tag="tri")
        nc.sync.dma_start(tri, ap["tri"])
        bqk = cp.tile([P, 4], F32, name="bqk", tag="bqk")
        nc.sync.dma_start(bqk, ap["bqk"])
        bv = cp.tile([1, HPC * VW], F16, name="bv", tag="bv")
        nc.sync.dma_start(bv, ap["bv"])
        ones1 = cp.tile([1, P], F16, name="ones1", tag="ones1")
        nc.sync.dma_start(ones1, ap["ones1"])
        for c in range(1, NSQ):
            nc.sync.dma_start(xts[:, c], ap["xln"][:, c])

        wq = xw.tile([P, KT, CH], F16, name="wq", tag="wq")
        nc.scalar.dma_start(wq, ap["wq"].rearrange("p (k c) -> p k c", k=KT))
        wk = xw.tile([P, KT, CH], F16, name="wk", tag="wk")
        nc.scalar.dma_start(wk, ap["wk"].rearrange("p (k c) -> p k c", k=KT))
        wv = xw.tile([P, KT, HPC * VW], F16, name="wv", tag="wv")
        nc.scalar.dma_start(wv, ap["wv"].rearrange("p (k c) -> p k c", k=KT))
        wp = xw.tile([P, 2, D], F16, name="wp", tag="wp")
        nc.scalar.dma_start(wp, ap["wp"].rearrange("p (k c) -> p k c", k=2))

        wq_t = [wq[:, k, :] for k in range(KT)]
        wk_t = [wk[:, k, :] for k in range(KT)]
        wv_t = [wv[:, k, :] for k in range(KT)]
        wp_t = [wp[:, k, :] for k in range(2)]

        # ---- activations living across phases ----
        qT = [acts.tile([P, S], F16, name=f"qT{i}", tag=f"qT{i}") for i in range(2)]
        kTt = [acts.tile([P, S], F16, name=f"kT{i}", tag=f"kT{i}") for i in range(2)]
        vv = acts.tile([P, NST, HPC * VW], F16, name="vv", tag="vv")
        outT = [acts.tile([P, S], F16, name=f"oT{i}", tag=f"oT{i}") for i in range(2)]

        def qkv_qk_group(c, dst, wt, bcol, i):
            ps = psA.tile([P, SQ], F32, name="ps", tag="ps")
            for k in range(KT):
                nc.tensor.matmul(
                    ps,
                    wt[k][:, i * P:(i + 1) * P],
                    xts[:, c, k, :],
                    start=(k == 0),
                    stop=(k == KT - 1),
                )
            with nc.allow_low_precision(reason="fp16 matmul inputs"):
                nc.vector.tensor_scalar_add(
                    dst[i][:, c * SQ:(c + 1) * SQ], ps, bqk[:, bcol + i:bcol + i + 1],
                )

        def qkv_v_group(st):
            # v rows (natural layout + interleaved ones cols)
            ps = psA.tile([P, SQ], F32, name="psv", tag="ps")
            psv = ps[:, 0:HPC * VW]
            for k in range(KT):
                nc.tensor.matmul(
                    psv,
                    xts[:, st // 4, k, (st % 4) * P:(st % 4 + 1) * P],
                    wv_t[k],
                    start=(k == 0),
                    stop=False,
                )
            # += ones_col(seq) x (bv | interleaved 1.0): v-bias + ones col
            nc.tensor.matmul(psv, ones1, bv, start=False, stop=True)
            with nc.allow_low_precision(reason="fp16 matmul inputs"):
                nc.vector.tensor_copy(vv[:, st, :], psv)

        def qkv_groups(c):
            for dst, wt, bcol in ((qT, wq_t, 0), (kTt, wk_t, 2)):
                for i in range(2):
                    yield lambda dst=dst, wt=wt, bcol=bcol, i=i: \
                        qkv_qk_group(c, dst, wt, bcol, i)
            for st in range(4 * c, 4 * c + 4):
                yield lambda st=st: qkv_v_group(st)

        def attention_pair(i, c, fillers=(), after_scores=None):
            """Heads 2i (kT/qT rows 0:64) and 2i+1 (rows 64:128) together.

            Both heads' scores for a key tile land in one 2-bank PSUM tile so
            a single exp instruction covers them (halves ScalarE instruction
            count). All scores are emitted before all attnv matmuls: the PE
            stream is in-order, so this keeps the PE on scores while
            ScalarE's exps pipeline behind. Diagonal key tiles are processed
            FIRST so their exp->mask chain completes before attnv needs them;
            the mask multiply runs on GpSimd (idle), not Vector."""
            nkt = 4 * (c + 1)
            # diagonal tiles first, then off-diagonal
            kts = list(range(4 * c, nkt)) + list(range(0, 4 * c))
            accs = [psB.tile([VW, SQ], F32, name="acc", tag="acc")
                    for _ in range(2)]

            exs = []
            for kt in kts:
                colo = max(0, kt * P - c * SQ)
                diag = colo > 0 or kt * P == c * SQ
                sc2 = psC.tile([P, 2, SQ], F32, name="sc2", tag="sc")
                for j in range(2):
                    ro = j * 64
                    nc.tensor.matmul(
                        sc2[:, j, colo:SQ],
                        kTt[i][ro:ro + 64, kt * P:(kt + 1) * P],
                        qT[i][ro:ro + 64, c * SQ + colo:(c + 1) * SQ],
                        start=True,
                        stop=True,
                    )
                ex2 = exp_pool.tile([P, 2, SQ], F16, name="ex2", tag="ex")
                nc.scalar.activation(
                    ex2[:, :, colo:SQ], sc2[:, :, colo:SQ], AFT.Exp, scale=SCALE,
                )
                if diag:
                    # zero the masked triangle of the diagonal block; GpSimd
                    # is otherwise idle and this keeps Vector off the
                    # score->attnv chain entirely
                    nc.gpsimd.tensor_mul(
                        ex2[:, :, colo:colo + P],
                        ex2[:, :, colo:colo + P],
                        tri[:, None, :].broadcast_to([P, 2, P]),
                    )
                exs.append((ex2, kt, colo))
            if after_scores is not None:
                after_scores()
            fillers = list(fillers)
            nf = len(fillers)
            for idx, (ex2, kt, colo) in enumerate(exs):
                for j in range(2):
                    h = 2 * i + j
                    nc.tensor.matmul(
                        accs[j][:, colo:SQ],
                        vv[:, kt, h * VW:(h + 1) * VW],
                        ex2[:, j, colo:SQ],
                        start=(idx == 0),
                        stop=(idx == nkt - 1),
                    )
                # dense PE filler between exp-paced attnv groups
                while fillers and len(fillers) > nf * (nkt - 1 - idx) // nkt:
                    fillers.pop(0)()
            # dn copies first: the deferred norm matmuls depend on them
            dns = []
            for j in range(2):
                dn = dh_pool.tile([1, SQ], F16, name="dn", tag="dn")
                with nc.allow_low_precision(reason="fp16 matmul inputs"):
                    nc.vector.tensor_copy(dn, accs[j][64:65, :])
                dns.append(dn)
            for j in range(2):
                with nc.allow_low_precision(reason="fp16 matmul inputs"):
                    nc.vector.tensor_copy(
                        outT[i][j * 64:j * 64 + 64, c * SQ:(c + 1) * SQ],
                        accs[j][0:64, :],
                    )
            return dns

        def norm_pair(c, i, dns):
            # outT *= 1/denominator: broadcast denoms via K=1 matmuls, one
            # 128-lane fast reciprocal, one fp16 multiply (on GpSimd)
            db = psA.tile([P, SQ], F32, name="ps", tag="ps")
            nc.tensor.matmul(
                db[0:64, :], ones1[:, 0:64], dns[0],
                start=True, stop=True,
            )
            nc.tensor.matmul(
                db[64:P, :], ones1[:, 0:64], dns[1],
                start=True, stop=True,
            )
            rc32 = rc_pool.tile([P, SQ], F32, name="rc32", tag="rc32")
            nc.vector.reciprocal_approx_fast(rc32, db)
            with nc.allow_low_precision(reason="fp16 matmul inputs"):
                nc.gpsimd.tensor_mul(
                    outT[i][:, c * SQ:(c + 1) * SQ],
                    outT[i][:, c * SQ:(c + 1) * SQ],
                    rc32,
                )

        def proj_mtile(m):
            ob = osb.tile([P, D], F16, name="ob", tag="ob")
            for nch in range(2):
                ps = psA.tile([P, SQ], F32, name="ps", tag="ps")
                for kk in range(2):
                    nc.tensor.matmul(
                        ps,
                        outT[kk][:, m * P:(m + 1) * P],
                        wp_t[kk][:, nch * SQ:(nch + 1) * SQ],
                        start=(kk == 0),
                        stop=(kk == 1),
                    )
                with nc.allow_low_precision(reason="partial sums; host sums fp32"):
                    nc.vector.tensor_copy(ob[:, nch * SQ:(nch + 1) * SQ], ps)
            nc.sync.dma_start(ap["out"][m * P:(m + 1) * P, :], ob)

        # ---- chunk-pipelined main body ----
        # chunk 0 QKV runs as soon as chunk-0 x + weights stream in; the
        # PE's HAM cold-start window is spent on this real work.
        for g in qkv_groups(0):
            g()
        # attention(c) runs against qkv chunks emitted one chunk ahead.
        # Dense PE filler between heads: remaining qkv chunks during c=1,2
        # and the saved-up projection tiles of chunks 0-2 during c=3 (the
        # largest, most exp-bound chunk). Each pair's norm matmuls are
        # deferred past the next pair's scores so the in-order PE never
        # stalls on the Vector dn-copy latency.
        pending_norm = None
        for c in range(NSQ):
            nxt = list(qkv_groups(c + 1)) if c + 1 < NSQ else []
            for i in range(2):
                fillers = list(nxt[4 * i:4 * i + 4])
                if c == NSQ - 2 and i == 1:
                    # chunk-0 proj tiles are normalized by now; the c=2 pairs
                    # have a little spare exp-paced filler capacity
                    fillers += [lambda: proj_mtile(0), lambda: proj_mtile(1)]
                if c == NSQ - 1:
                    ms = range(2, 8) if i == 0 else range(8, 12)
                    fillers += [(lambda m=m: proj_mtile(m)) for m in ms]
                dns = attention_pair(i, c, fillers, after_scores=pending_norm)
                pending_norm = (lambda c=c, i=i, dns=dns: norm_pair(c, i, dns))
        # tail: norm of the last pair, then the chunk-3 projection tiles
        pending_norm()
        for m in range(4 * (NSQ - 1), 4 * NSQ):
            proj_mtile(m)


def build_program():
    nc = bacc.Bacc("TRN2", target_bir_lowering=False, debug=False,
                   num_devices=NCORES)
    ap = {}
    for name, shape, dt in (
        ("xln", [P, NSQ, KT, SQ], F16),
        ("wq", [P, KT * CH], F16), ("wk", [P, KT * CH], F16),
        ("wv", [P, KT * HPC * VW], F16), ("wp", [P, 2 * D], F16),
        ("bqk", [P, 4], F32), ("bv", [1, HPC * VW], F16),
        ("tri", [P, P], F16), ("ones1", [1, P], F16),
    ):
        ap[name] = nc.dram_tensor(name, shape, dt, kind="ExternalInput").ap()
    ap["out"] = nc.dram_tensor("out", [S, D], F16, kind="ExternalOutput").ap()

    with tile.TileContext(nc) as tc:
        emit_kernel(nc, tc, ap)
    nc.compile()
    return nc


def make_core_inputs(hidden_states, w_attn, b_attn, w_proj):
    """Host-side sharding: per-core input dicts (core = batch*4 + head_group).

    All tensors are relayouted partition-major so every device DMA is a
    contiguous 128-line transfer (cheap HWDGE descriptor generation)."""
    f16, f32 = np.float16, np.float32
    x = np.asarray(hidden_states, f32)
    w_attn = np.asarray(w_attn, f32)
    b_attn = np.asarray(b_attn, f32)
    w_proj = np.asarray(w_proj, f32)

    tri = (np.arange(P)[:, None] <= np.arange(P)[None, :]).astype(f16)
    ones_row = np.ones((1, P), f16)

    def kmaj(w):  # [D, C] -> [P, KT*C] with w[k*P+p, c] at [p, k*C+c]
        kt = w.shape[0] // P
        return np.ascontiguousarray(
            w.reshape(kt, P, -1).transpose(1, 0, 2).reshape(P, -1)).astype(f16)

    # x[b][c*SQ+s, k*P+p] -> xln[p, c, k, s]
    xlns = [
        np.ascontiguousarray(
            x[b].reshape(NSQ, SQ, KT, P).transpose(3, 0, 2, 1)).astype(f16)
        for b in range(B)
    ]

    in_maps = []
    for core in range(NCORES):
        b, g = core // HPC, core % HPC
        wq = kmaj(w_attn[:, g * CH:(g + 1) * CH])
        wk = kmaj(w_attn[:, D + g * CH:D + (g + 1) * CH])
        wv_full = np.zeros((D, HPC * VW), f32)
        bv = np.zeros((1, HPC * VW), f16)
        for h in range(HPC):
            src = 2 * D + (g * HPC + h) * HD
            wv_full[:, h * VW:h * VW + HD] = w_attn[:, src:src + HD]
            bv[0, h * VW:h * VW + HD] = b_attn[src:src + HD]
            bv[0, h * VW + HD] = 1.0
        wv = kmaj(wv_full)
        bqk = np.zeros((P, 4), f32)
        bqk[:, 0:2] = b_attn[g * CH:(g + 1) * CH].reshape(2, P).T
        bqk[:, 2:4] = b_attn[D + g * CH:D + (g + 1) * CH].reshape(2, P).T
        wp = kmaj(w_proj[g * CH:(g + 1) * CH, :])
        in_maps.append({
            "xln": xlns[b], "wq": wq, "wk": wk, "wv": wv, "wp": wp,
            "bqk": bqk, "bv": bv, "tri": tri, "ones1": ones_row,
        })
    return in_maps


_PROGRAM = None


def kernel(hidden_states, w_attn, b_attn, w_proj, b_proj):
    global _PROGRAM
    if _PROGRAM is None:
        _PROGRAM = build_program()
    in_maps = make_core_inputs(hidden_states, w_attn, b_attn, w_proj)
    res = run_bass_kernel_spmd(_PROGRAM, in_maps, core_ids=list(range(NCORES)))
    out = np.zeros((B, S, D), np.float32)
    for core in range(NCORES):
        out[core // HPC] += res.results[core]["out"].astype(np.float32)
    out += np.asarray(b_proj, np.float32)
    return out


# revision 11
# speedup vs baseline: 1.1923x; 1.0208x over previous
"""Fused causal multi-head attention block on 8 Trainium2 NeuronCores.

Problem (GPT-2 style attention, B=2, S=2048, D=1024, H=16, hd=64):
    qkv = x @ w_attn + b_attn ; split q,k,v ; per-head causal softmax(q k^T / 8) v
    out = attn_out @ w_proj + b_proj

Sharding: data parallel on batch (2) x tensor parallel on heads (4 groups of 4
heads). Core c -> batch c//4, head group c%4. Each core computes a partial
[S, D] output (its heads' slice of w_proj rows); host sums the 4 partials per
batch and adds b_proj.

Per-core kernel layout tricks:
- scores are computed TRANSPOSED (scoresT[key, query]) so the softmax
  denominator falls out of the attn@v matmul by appending a ones-column to v:
  [v | 1]^T @ exp(scoresT) yields the unnormalized output and the per-query
  denominator in one PSUM accumulation.
- matmul inputs are fp16 (full PE rate + fast weight loads); all accumulation
  is fp32 in PSUM. exp(s/8) is in [0, ~13], well inside fp16 range.
- causal masking: fully-masked blocks are skipped via restricted matmul
  widths; diagonal blocks get their exp output multiplied by a 0/1 triangle
  on the (otherwise idle) GpSimd engine, keeping Vector free for psum copies.
- all DRAM tensors are host-relayouted to partition-major contiguous form so
  every DMA issue is a cheap 128-line descriptor; x is loaded per 512-query
  chunk (chunk 0 first) and weights stream in parallel on the Scalar HWDGE
  ring, so real QKV work starts ~8us in with no junk warmup needed.
- emission is chunk-pipelined (QKV chunk c, attention chunk c, projection
  chunk c) so the PE always has dense matmul work while ScalarE runs exp;
  each pair's normalization matmuls are deferred past the next pair's score
  matmuls so the in-order PE never waits on Vector/GpSimd latency.
"""

import sys

sys.path.insert(0, "/opt/trn_rl_repo")

import numpy as np

import concourse.bass as bass
import concourse.mybir as mybir
import concourse.tile as tile
from concourse import bacc
from concourse.bass_utils import run_bass_kernel_spmd

F32 = mybir.dt.float32
F16 = mybir.dt.float16
AFT = mybir.ActivationFunctionType

B, S, D, H, HD = 2, 2048, 1024, 16, 64
NCORES = 8
HPC = 4            # heads per core
CH = HPC * HD      # 256 channels per core
VW = HD + 1        # v width incl. ones column
P = 128
KT = D // P        # 8 contraction tiles over D
SQ = 512           # query/N chunk
NSQ = S // SQ      # 4
NST = S // P       # 16 seq tiles
SCALE = 1.0 / np.sqrt(HD)


def emit_kernel(nc, tc, ap):
    """Emit the per-core program. `ap` is a dict of DRAM APs."""
    with (
        tc.tile_pool(name="const", bufs=1) as cp,
        tc.tile_pool(name="xw", bufs=1) as xw,
        tc.tile_pool(name="act", bufs=1) as acts,
        tc.tile_pool(name="ex", bufs=20) as exp_pool,
        tc.tile_pool(name="dh", bufs=4) as dh_pool,
        tc.tile_pool(name="rc", bufs=2) as rc_pool,
        tc.tile_pool(name="osb", bufs=3) as osb,
        tc.tile_pool(name="psA", bufs=2, space="PSUM") as psA,
        tc.tile_pool(name="psB", bufs=2, space="PSUM") as psB,
        tc.tile_pool(name="psC", bufs=2, space="PSUM") as psC,
    ):
        # ---- input DMAs. Two parallel HWDGE rings: x chunks + small consts
        # on Sync, weights on Scalar. Chunk-0 x and wq are split in halves so
        # the first QKV matmuls can start after ~0.75MB instead of ~1.5MB.
        xts = xw.tile([P, NSQ, KT, SQ], F16, name="xts", tag="xts")
        half = KT // 2
        nc.sync.dma_start(xts[:, 0, 0:half], ap["xln"][:, 0, 0:half])
        nc.sync.dma_start(xts[:, 0, half:KT], ap["xln"][:, 0, half:KT])
        tri = cp.tile([P, P], F16, name="tri", tag="tri")
        nc.sync.dma_start(tri, ap["tri"])
        bqk = cp.tile([P, 4], F32, name="bqk", tag="bqk")
        nc.sync.dma_start(bqk, ap["bqk"])
        bv = cp.tile([1, HPC * VW], F16, name="bv", tag="bv")
        nc.sync.dma_start(bv, ap["bv"])
        ones1 = cp.tile([1, P], F16, name="ones1", tag="ones1")
        nc.sync.dma_start(ones1, ap["ones1"])
        for c in range(1, NSQ):
            nc.sync.dma_start(xts[:, c], ap["xln"][:, c])

        wq = xw.tile([P, KT, CH], F16, name="wq", tag="wq")
        wq_ap = ap["wq"].rearrange("p (k c) -> p k c", k=KT)
        nc.scalar.dma_start(wq[:, 0:half], wq_ap[:, 0:half])
        nc.scalar.dma_start(wq[:, half:KT], wq_ap[:, half:KT])
        wk = xw.tile([P, KT, CH], F16, name="wk", tag="wk")
        nc.scalar.dma_start(wk, ap["wk"].rearrange("p (k c) -> p k c", k=KT))
        wv = xw.tile([P, KT, HPC * VW], F16, name="wv", tag="wv")
        nc.scalar.dma_start(wv, ap["wv"].rearrange("p (k c) -> p k c", k=KT))
        wp = xw.tile([P, 2, D], F16, name="wp", tag="wp")
        nc.scalar.dma_start(wp, ap["wp"].rearrange("p (k c) -> p k c", k=2))

        wq_t = [wq[:, k, :] for k in range(KT)]
        wk_t = [wk[:, k, :] for k in range(KT)]
        wv_t = [wv[:, k, :] for k in range(KT)]
        wp_t = [wp[:, k, :] for k in range(2)]

        # ---- activations living across phases ----
        qT = [acts.tile([P, S], F16, name=f"qT{i}", tag=f"qT{i}") for i in range(2)]
        kTt = [acts.tile([P, S], F16, name=f"kT{i}", tag=f"kT{i}") for i in range(2)]
        vv = acts.tile([P, NST, HPC * VW], F16, name="vv", tag="vv")
        outT = [acts.tile([P, S], F16, name=f"oT{i}", tag=f"oT{i}") for i in range(2)]

        def qkv_qk_group(c, dst, wt, bcol, i):
            ps = psA.tile([P, SQ], F32, name="ps", tag="ps")
            for k in range(KT):
                nc.tensor.matmul(
                    ps,
                    wt[k][:, i * P:(i + 1) * P],
                    xts[:, c, k, :],
                    start=(k == 0),
                    stop=(k == KT - 1),
                )
            with nc.allow_low_precision(reason="fp16 matmul inputs"):
                nc.vector.tensor_scalar_add(
                    dst[i][:, c * SQ:(c + 1) * SQ], ps, bqk[:, bcol + i:bcol + i + 1],
                )

        def qkv_v_group(st):
            # v rows (natural layout + interleaved ones cols)
            ps = psA.tile([P, SQ], F32, name="psv", tag="ps")
            psv = ps[:, 0:HPC * VW]
            for k in range(KT):
                nc.tensor.matmul(
                    psv,
                    xts[:, st // 4, k, (st % 4) * P:(st % 4 + 1) * P],
                    wv_t[k],
                    start=(k == 0),
                    stop=False,
                )
            # += ones_col(seq) x (bv | interleaved 1.0): v-bias + ones col
            nc.tensor.matmul(psv, ones1, bv, start=False, stop=True)
            with nc.allow_low_precision(reason="fp16 matmul inputs"):
                nc.vector.tensor_copy(vv[:, st, :], psv)

        def qkv_groups(c):
            for dst, wt, bcol in ((qT, wq_t, 0), (kTt, wk_t, 2)):
                for i in range(2):
                    yield lambda dst=dst, wt=wt, bcol=bcol, i=i: \
                        qkv_qk_group(c, dst, wt, bcol, i)
            for st in range(4 * c, 4 * c + 4):
                yield lambda st=st: qkv_v_group(st)

        def make_scores(i, c):
            """Score matmul + exp emission closures for pair (c, i), one per
            key tile. Both heads' scores for a key tile land in one 2-bank
            PSUM tile so a single exp instruction covers them. Diagonal key
            tiles go FIRST so their exp->mask chain (mask on the idle GpSimd)
            completes long before attnv consumes them. The closures are
            interleaved into the PREVIOUS pair's attnv phase so ScalarE's exp
            stream never starves at pair boundaries."""
            nkt = 4 * (c + 1)
            kts = list(range(4 * c, nkt)) + list(range(0, 4 * c))
            exs = []

            def emit_kt(kt):
                colo = max(0, kt * P - c * SQ)
                diag = colo > 0 or kt * P == c * SQ
                sc2 = psC.tile([P, 2, SQ], F32, name="sc2", tag="sc")
                for j in range(2):
                    ro = j * 64
                    nc.tensor.matmul(
                        sc2[:, j, colo:SQ],
                        kTt[i][ro:ro + 64, kt * P:(kt + 1) * P],
                        qT[i][ro:ro + 64, c * SQ + colo:(c + 1) * SQ],
                        start=True,
                        stop=True,
                    )
                ex2 = exp_pool.tile([P, 2, SQ], F16, name="ex2", tag="ex")
                nc.scalar.activation(
                    ex2[:, :, colo:SQ], sc2[:, :, colo:SQ], AFT.Exp, scale=SCALE,
                )
                if diag:
                    nc.gpsimd.tensor_mul(
                        ex2[:, :, colo:colo + P],
                        ex2[:, :, colo:colo + P],
                        tri[:, None, :].broadcast_to([P, 2, P]),
                    )
                exs.append((ex2, kt, colo))

            return [(lambda kt=kt: emit_kt(kt)) for kt in kts], exs

        def do_attnv(i, c, exs, fillers, next_scores, dn_first=False):
            """attnv accumulation for pair (c, i), interleaved with the next
            pair's score/exp emissions (to keep ScalarE saturated) and dense
            PE fillers (qkv/proj work)."""
            nkt = 4 * (c + 1)
            fillers = list(fillers)
            next_scores = list(next_scores)
            nf, ns = len(fillers), len(next_scores)
            accs = [psB.tile([VW, SQ], F32, name="acc", tag="acc")
                    for _ in range(2)]
            # pre-work before the first attnv matmul: covers the PSUM
            # acc-slot WAR on the previous pair's psum->sbuf copies and
            # jump-starts the next pair's exp stream
            for pre in range(2):
                if fillers:
                    fillers.pop(0)()
                if next_scores:
                    next_scores.pop(0)()
            for idx, (ex2, kt, colo) in enumerate(exs):
                for j in range(2):
                    h = 2 * i + j
                    nc.tensor.matmul(
                        accs[j][:, colo:SQ],
                        vv[:, kt, h * VW:(h + 1) * VW],
                        ex2[:, j, colo:SQ],
                        start=(idx == 0),
                        stop=(idx == nkt - 1),
                    )
                while next_scores and \
                        (ns - len(next_scores)) < ns * (idx + 1) // nkt:
                    next_scores.pop(0)()
                while fillers and len(fillers) > nf * (nkt - 1 - idx) // nkt:
                    fillers.pop(0)()
            # psum->sbuf copies; outT first so the next pair's attnv can
            # reuse the acc psum slots ASAP (dn first at the tail, where the
            # norm chain is the critical path instead)
            dns = []
            if dn_first:
                for j in range(2):
                    dn = dh_pool.tile([1, SQ], F16, name="dn", tag="dn")
                    with nc.allow_low_precision(reason="fp16 matmul inputs"):
                        nc.vector.tensor_copy(dn, accs[j][64:65, :])
                    dns.append(dn)
            for j in range(2):
                with nc.allow_low_precision(reason="fp16 matmul inputs"):
                    nc.vector.tensor_copy(
                        outT[i][j * 64:j * 64 + 64, c * SQ:(c + 1) * SQ],
                        accs[j][0:64, :],
                    )
            if not dn_first:
                for j in range(2):
                    dn = dh_pool.tile([1, SQ], F16, name="dn", tag="dn")
                    with nc.allow_low_precision(reason="fp16 matmul inputs"):
                        nc.vector.tensor_copy(dn, accs[j][64:65, :])
                    dns.append(dn)
            return dns

        def norm_pair(c, i, dns, on_vector=False):
            # outT *= 1/denominator: broadcast denoms via K=1 matmuls, one
            # 128-lane fast reciprocal, one fp16 multiply (on GpSimd
            # mid-kernel; on Vector at the tail where latency matters)
            db = psA.tile([P, SQ], F32, name="ps", tag="ps")
            nc.tensor.matmul(
                db[0:64, :], ones1[:, 0:64], dns[0],
                start=True, stop=True,
            )
            nc.tensor.matmul(
                db[64:P, :], ones1[:, 0:64], dns[1],
                start=True, stop=True,
            )
            rc32 = rc_pool.tile([P, SQ], F32, name="rc32", tag="rc32")
            nc.vector.reciprocal_approx_fast(rc32, db)
            eng = nc.vector if on_vector else nc.gpsimd
            with nc.allow_low_precision(reason="fp16 matmul inputs"):
                eng.tensor_mul(
                    outT[i][:, c * SQ:(c + 1) * SQ],
                    outT[i][:, c * SQ:(c + 1) * SQ],
                    rc32,
                )

        def proj_mtile(m, split_dma=False):
            ob = osb.tile([P, D], F16, name="ob", tag="ob")
            for nch in range(2):
                ps = psA.tile([P, SQ], F32, name="ps", tag="ps")
                for kk in range(2):
                    nc.tensor.matmul(
                        ps,
                        outT[kk][:, m * P:(m + 1) * P],
                        wp_t[kk][:, nch * SQ:(nch + 1) * SQ],
                        start=(kk == 0),
                        stop=(kk == 1),
                    )
                with nc.allow_low_precision(reason="partial sums; host sums fp32"):
                    nc.vector.tensor_copy(ob[:, nch * SQ:(nch + 1) * SQ], ps)
                if split_dma:
                    nc.sync.dma_start(
                        ap["out"][m * P:(m + 1) * P, nch * SQ:(nch + 1) * SQ],
                        ob[:, nch * SQ:(nch + 1) * SQ],
                    )
            if not split_dma:
                nc.sync.dma_start(ap["out"][m * P:(m + 1) * P, :], ob)

        # ---- chunk-pipelined main body ----
        # chunk 0 QKV runs as soon as chunk-0 x + weights stream in; the
        # PE's HAM cold-start window is spent on this real work.
        for g in qkv_groups(0):
            g()
        # Pair p's scores/exps are interleaved into pair p-1's attnv phase
        # so both the PE and ScalarE stay saturated; qkv chunk c+1 and the
        # ready projection tiles serve as dense PE filler inside the
        # exp-paced attnv windows. Each pair's norm is deferred two slots
        # into the NEXT pair's filler stream (past the Vector dn-copies).
        pairs = [(c, i) for c in range(NSQ) for i in range(2)]
        cl0, cur_exs = make_scores(0, 0)
        for cl in cl0:
            cl()
        pending_norm = None
        for pidx, (c, i) in enumerate(pairs):
            last = pidx == len(pairs) - 1
            if not last:
                nxt_c, nxt_i = pairs[pidx + 1]
                nxt_cl, nxt_exs = make_scores(nxt_i, nxt_c)
            else:
                nxt_cl, nxt_exs = [], None
            fillers = []
            if c + 1 < NSQ:
                nxt_qkv = list(qkv_groups(c + 1))
                fillers += nxt_qkv[4 * i:4 * i + 4]
            if c == NSQ - 2 and i == 1:
                fillers += [lambda: proj_mtile(0), lambda: proj_mtile(1)]
            if c == NSQ - 1:
                ms = range(2, 8) if i == 0 else range(8, 12)
                fillers += [(lambda m=m: proj_mtile(m)) for m in ms]
            if pending_norm is not None:
                fillers.insert(2, pending_norm)
            dns = do_attnv(i, c, cur_exs, fillers, nxt_cl, dn_first=last)
            pending_norm = (lambda c=c, i=i, dns=dns: norm_pair(c, i, dns))
            cur_exs = nxt_exs
        # tail: norm of the last pair (mul on Vector: latency-critical),
        # then the chunk-3 projection tiles; the very last output DMA is
        # split in halves so its issue overlaps the second CAST
        norm_pair(NSQ - 1, 1, dns, on_vector=True)
        for m in range(4 * (NSQ - 1), 4 * NSQ):
            proj_mtile(m, split_dma=(m == 4 * NSQ - 1))


def build_program():
    nc = bacc.Bacc("TRN2", target_bir_lowering=False, debug=False,
                   num_devices=NCORES)
    ap = {}
    for name, shape, dt in (
        ("xln", [P, NSQ, KT, SQ], F16),
        ("wq", [P, KT * CH], F16), ("wk", [P, KT * CH], F16),
        ("wv", [P, KT * HPC * VW], F16), ("wp", [P, 2 * D], F16),
        ("bqk", [P, 4], F32), ("bv", [1, HPC * VW], F16),
        ("tri", [P, P], F16), ("ones1", [1, P], F16),
    ):
        ap[name] = nc.dram_tensor(name, shape, dt, kind="ExternalInput").ap()
    ap["out"] = nc.dram_tensor("out", [S, D], F16, kind="ExternalOutput").ap()

    with tile.TileContext(nc) as tc:
        emit_kernel(nc, tc, ap)
    nc.compile()
    return nc


def make_core_inputs(hidden_states, w_attn, b_attn, w_proj):
    """Host-side sharding: per-core input dicts (core = batch*4 + head_group).

    All tensors are relayouted partition-major so every device DMA is a
    contiguous 128-line transfer (cheap HWDGE descriptor generation)."""
    f16, f32 = np.float16, np.float32
    x = np.asarray(hidden_states, f32)
    w_attn = np.asarray(w_attn, f32)
    b_attn = np.asarray(b_attn, f32)
    w_proj = np.asarray(w_proj, f32)

    tri = (np.arange(P)[:, None] <= np.arange(P)[None, :]).astype(f16)
    ones_row = np.ones((1, P), f16)

    def kmaj(w):  # [D, C] -> [P, KT*C] with w[k*P+p, c] at [p, k*C+c]
        kt = w.shape[0] // P
        return np.ascontiguousarray(
            w.reshape(kt, P, -1).transpose(1, 0, 2).reshape(P, -1)).astype(f16)

    # x[b][c*SQ+s, k*P+p] -> xln[p, c, k, s]
    xlns = [
        np.ascontiguousarray(
            x[b].reshape(NSQ, SQ, KT, P).transpose(3, 0, 2, 1)).astype(f16)
        for b in range(B)
    ]

    in_maps = []
    for core in range(NCORES):
        b, g = core // HPC, core % HPC
        wq = kmaj(w_attn[:, g * CH:(g + 1) * CH])
        wk = kmaj(w_attn[:, D + g * CH:D + (g + 1) * CH])
        wv_full = np.zeros((D, HPC * VW), f32)
        bv = np.zeros((1, HPC * VW), f16)
        for h in range(HPC):
            src = 2 * D + (g * HPC + h) * HD
            wv_full[:, h * VW:h * VW + HD] = w_attn[:, src:src + HD]
            bv[0, h * VW:h * VW + HD] = b_attn[src:src + HD]
            bv[0, h * VW + HD] = 1.0
        wv = kmaj(wv_full)
        bqk = np.zeros((P, 4), f32)
        bqk[:, 0:2] = b_attn[g * CH:(g + 1) * CH].reshape(2, P).T
        bqk[:, 2:4] = b_attn[D + g * CH:D + (g + 1) * CH].reshape(2, P).T
        wp = kmaj(w_proj[g * CH:(g + 1) * CH, :])
        in_maps.append({
            "xln": xlns[b], "wq": wq, "wk": wk, "wv": wv, "wp": wp,
            "bqk": bqk, "bv": bv, "tri": tri, "ones1": ones_row,
        })
    return in_maps


_PROGRAM = None


def kernel(hidden_states, w_attn, b_attn, w_proj, b_proj):
    global _PROGRAM
    if _PROGRAM is None:
        _PROGRAM = build_program()
    in_maps = make_core_inputs(hidden_states, w_attn, b_attn, w_proj)
    res = run_bass_kernel_spmd(_PROGRAM, in_maps, core_ids=list(range(NCORES)))
    out = np.zeros((B, S, D), np.float32)
    for core in range(NCORES):
        out[core // HPC] += res.results[core]["out"].astype(np.float32)
    out += np.asarray(b_proj, np.float32)
    return out
